# revision 4
# baseline (speedup 1.0000x reference)
"""Self-contained 8-core Trainium2 Bass kernel for the 2-layer GAT problem.

Strategy
--------
* Sort edges by dst; partition edge list across 8 cores at dst-node
  boundaries, so every dst node's incoming edges live on exactly one core.
  Edge softmax and aggregation are then fully core-local (no cross-core
  segment reductions at all).
* Phase A (per core): feat1 = x @ W1 for the core's node block via
  host-transposed x (stationary lhsT chunks). el1/er1 come from the same
  stationary operand with host-precomputed WA = W1 @ blockdiag(a).
  Results go to a bf16 row table [feat1 | el1], AllGathered so every core
  can gather arbitrary source rows.
* Phase B: per dst-window (127 dst nodes + 1 trash row), dma_gather the
  per-edge source rows (int16 indices; the table is split in lo/hi halves
  so indices fit int16), compute ex = exp(leakyrelu(el_src + er_dst))
  (no segment max needed: values are small, and any per-segment shift
  cancels in the softmax), scale rows by ex, and aggregate with a one-hot
  dst mask via the tensor engine into PSUM.  The mask's extra column trick
  accumulates the softmax denominator in the same pass.
* Phase C: h = elu(rst/s + b1); feat2/el2/er2/resid via PE-transposed h.
* Phase D: same as B for layer 2 (single head; ex folds into the mask).
* Output rows are written per-core and concatenated on the host.

The container's walrus rejects instructions with >1 sync-wait, and plain
Bass+Tile never inserts GPSIMD library loads — `_finalize` patches both.
"""

import math
import numpy as np
import ml_dtypes

import concourse.bass as bass
import concourse.mybir as mybir
import concourse.tile as tile
from concourse.bass_utils import run_bass_kernel_spmd
from concourse.masks import make_identity

FP32 = mybir.dt.float32
BF16 = mybir.dt.bfloat16
I16 = mybir.dt.int16

NCORES = 8
P = 128
NEG_SLOPE = 0.2

# problem dims (hardcoded per contract)
IN_DIM = 256
HIDDEN = 64
HEADS1 = 4
F1 = HEADS1 * HIDDEN          # 256
N_CLASSES = 64
F2 = N_CLASSES                # 64 (single head)
T1_COLS = 384                 # [feat1 bf16 256 | el1 bf16 4 | pad] -> 768B rows
T2_COLS = 128                 # [feat2 bf16 64 | el2 bf16 1 | pad] -> 256B rows
ER_COLS = 64                  # [er1 f32 x4 | er2 f32 x1 | pad] -> 256B rows


# --------------------------------------------------------------------------
# finalize passes: make Tile output compatible with this container's walrus
# --------------------------------------------------------------------------

def _insert_library_loads(nc):
    import bass_rust as _bass_rust
    from concourse.library_config import all_libraries, standard

    mask = {}
    for lib in all_libraries:
        for t in lib.instructions:
            mask[t] = mask.get(t, 0) | (1 << lib.index)
    _bass_rust.insert_library_loads(nc, mask, len(all_libraries), standard.index)


def _split_multi_waits(nc, max_waits=1):
    n = 0
    for bb in nc.m.functions[0].blocks:
        insts = bb.instructions
        if not any(i.sync_info and i.sync_info.on_wait
                   and len(i.sync_info.on_wait) > max_waits for i in insts):
            continue
        out = []
        for inst in insts:
            si = inst.sync_info
            if si and si.on_wait and len(si.on_wait) > max_waits:
                waits = list(si.on_wait)
                extra, keep = waits[:-max_waits], waits[-max_waits:]
                for j in range(0, len(extra), max_waits):
                    nop = mybir.InstNoOp(
                        name=nc.get_next_instruction_name(),
                        engine=inst.engine,
                        sync_info=mybir.SyncInfo(
                            on_wait=extra[j:j + max_waits], on_update=[]),
                        bass_nofuse=True)
                    nc.register_instruction(nop)
                    out.append(nop)
                    n += 1
                si.on_wait = keep
            out.append(inst)
        bb.instructions = out
    return n


def _finalize(nc):
    _insert_library_loads(nc)
    _split_multi_waits(nc)
    mybir.codegen_inst_isa_subclasses(nc)


# --------------------------------------------------------------------------
# host-side graph preprocessing
# --------------------------------------------------------------------------

def _wrap16(idx, ncols):
    """Wrap a 1-D index list into the [128, ncols] int16 layout dma_gather
    expects: index i at [16g + i%16, i//16] for all 8 groups g."""
    arr = np.zeros((P, ncols), np.int16)
    n = len(idx)
    cols = (n + 15) // 16
    block = np.zeros((16, ncols), np.int16)
    pad = np.zeros(cols * 16 - n, np.int16)
    w = np.concatenate([idx.astype(np.int16), pad]).reshape(cols, 16).T
    block[:, :cols] = w
    for g in range(8):
        arr[16 * g:16 * g + 16, :] = block
    return arr


def _prep_graph(src, dst, n_nodes):
    """Edge partitioning + per-core gather/mask layouts."""
    E = len(src)
    order = np.argsort(dst, kind="stable")
    ds = dst[order].astype(np.int64)
    ss = src[order].astype(np.int64)

    counts = np.bincount(ds, minlength=n_nodes)
    cum = np.concatenate([[0], np.cumsum(counts)])  # cum[n] = #edges dst<n

    bounds = [0]
    for i in range(1, NCORES):
        n = int(np.searchsorted(cum, round(E * i / NCORES)))
        n = min(max(n, bounds[-1] + 1), n_nodes - (NCORES - i))
        bounds.append(n)
    bounds.append(n_nodes)
    nlocs = [bounds[i + 1] - bounds[i] for i in range(NCORES)]
    NMAX = max(nlocs)
    NW = (NMAX + 126) // 127
    NR = P * ((NW * 127 + P - 1) // P)
    HALF = (NCORES // 2) * NR

    rank_of = np.searchsorted(bounds, np.arange(n_nodes), side="right") - 1
    base_of = np.asarray(bounds[:-1])[rank_of]
    table_row = rank_of * NR + (np.arange(n_nodes) - base_of)  # node -> table row

    # first pass: per (core, window) lo/hi edge lists -> K_LO/K_HI
    win_edges = []  # [core][w] -> (lo_rows, hi_rows, lo_dstwin, hi_dstwin, lo_dloc, hi_dloc)
    K_LO = K_HI = 1
    for c in range(NCORES):
        n0, n1 = bounds[c], bounds[c + 1]
        rows_c = []
        for w in range(NW):
            wn0 = n0 + 127 * w
            wn1 = min(wn0 + 127, n1)
            if wn0 >= n1:
                e0 = e1 = 0
                eds = ess = np.zeros(0, np.int64)
            else:
                e0, e1 = cum[wn0], cum[wn1]
                eds, ess = ds[e0:e1], ss[e0:e1]
            trow = table_row[ess] if len(ess) else np.zeros(0, np.int64)
            lo = trow < HALF
            lo_rows = trow[lo]
            hi_rows = trow[~lo] - HALF
            lo_dwin = (eds[lo] - wn0) if len(eds) else np.zeros(0, np.int64)
            hi_dwin = (eds[~lo] - wn0) if len(eds) else np.zeros(0, np.int64)
            lo_dloc = (eds[lo] - n0) if len(eds) else np.zeros(0, np.int64)
            hi_dloc = (eds[~lo] - n0) if len(eds) else np.zeros(0, np.int64)
            rows_c.append((lo_rows, hi_rows, lo_dwin, hi_dwin, lo_dloc, hi_dloc))
            K_LO = max(K_LO, (len(lo_rows) + P - 1) // P)
            K_HI = max(K_HI, (len(hi_rows) + P - 1) // P)
        win_edges.append(rows_c)

    KT = K_LO + K_HI
    idxm = np.zeros((NCORES, P, NW * KT * 8), np.int16)
    idxe = np.zeros((NCORES, P, NW * KT * 8), np.int16)
    dstw = np.full((NCORES, P, NW * KT), 127.0, np.float32)

    for c in range(NCORES):
        for w in range(NW):
            lo_rows, hi_rows, lo_dwin, hi_dwin, lo_dloc, hi_dloc = win_edges[c][w]
            col0 = w * KT * 8
            # main gather calls
            idxm[c][:, col0:col0 + K_LO * 8] = _wrap16(
                np.pad(lo_rows, (0, K_LO * P - len(lo_rows))), K_LO * 8)
            idxm[c][:, col0 + K_LO * 8:col0 + KT * 8] = _wrap16(
                np.pad(hi_rows, (0, K_HI * P - len(hi_rows))), K_HI * 8)
            # er gather: one call covering all KT*128 slots in slot order
            er_idx = np.zeros(KT * P, np.int64)
            er_idx[:len(lo_dloc)] = lo_dloc
            er_idx[K_LO * P:K_LO * P + len(hi_dloc)] = hi_dloc
            idxe[c][:, col0:col0 + KT * 8] = _wrap16(er_idx, KT * 8)
            # per-slot in-window dst (127 = trash for pads)
            dw = np.full(KT * P, 127, np.int64)
            dw[:len(lo_dwin)] = lo_dwin
            dw[K_LO * P:K_LO * P + len(hi_dwin)] = hi_dwin
            s = np.arange(KT * P)
            dstw[c][s % P, w * KT + s // P] = dw

    meta = dict(NW=NW, NR=NR, KT=KT, K_LO=K_LO, K_HI=K_HI, HALF=HALF,
                bounds=bounds, nlocs=nlocs)
    return meta, idxm, idxe, dstw


# --------------------------------------------------------------------------
# device program
# --------------------------------------------------------------------------

def _gather_chunks(nc, out_tile, in_ap, idx_sb, col0, t0, ntiles, elem, reg8, regs):
    """Issue dma_gather calls of at most 8 tiles (1024 idx) each."""
    done = 0
    while done < ntiles:
        k = min(8, ntiles - done)
        nc.gpsimd.dma_gather(
            out_ap=out_tile[:, t0 + done:t0 + done + k, :], in_ap=in_ap,
            idxs_ap=idx_sb[:, col0 + (t0 + done) * 8:col0 + (t0 + done + k) * 8],
            num_idxs=k * P, num_idxs_reg=(reg8 if k == 8 else regs[k]),
            elem_size=elem)
        done += k


def _build(meta):
    import os
    PHASES = int(os.environ.get("GAT_PHASES", "5"))
    BSUB = int(os.environ.get("GAT_BSUB", "4"))
    REPS = int(os.environ.get("GAT_REPS", "1"))
    SIM = int(os.environ.get("GAT_SIM", "0"))
    NW, NR, KT, K_LO = meta["NW"], meta["NR"], meta["KT"], meta["K_LO"]
    NA = NR // P

    nc = bass.Bass(num_devices=NCORES)
    xt = nc.declare_dram_parameter("xt", [2, P, NR], FP32, isOutput=False)
    wp1 = nc.declare_dram_parameter("wp1", [2, P, F1 + 8], FP32, isOutput=False)
    wp2 = nc.declare_dram_parameter("wp2", [2, P, F2 + 2 + F2], FP32, isOutput=False)
    iota_in = nc.declare_dram_parameter("iota", [P, P], BF16, isOutput=False)
    b1_in = nc.declare_dram_parameter("b1bc", [P, F1], FP32, isOutput=False)
    b2_in = nc.declare_dram_parameter("b2bc", [P, F2], FP32, isOutput=False)
    idxm_in = nc.declare_dram_parameter("idxm", [P, NW * KT * 8], I16, isOutput=False)
    idxe_in = nc.declare_dram_parameter("idxe", [P, NW * KT * 8], I16, isOutput=False)
    dstw_in = nc.declare_dram_parameter("dstw", [P, NW * KT], FP32, isOutput=False)
    out_loc = nc.declare_dram_parameter("out", [NR, F2], FP32, isOutput=True)

    t1_loc = nc.dram_tensor("t1_loc", [NR, T1_COLS], BF16)
    t1_full = nc.dram_tensor("t1_full", [NCORES * NR, T1_COLS], BF16, addr_space="Shared")
    t2_loc = nc.dram_tensor("t2_loc", [NR, T2_COLS], BF16)
    t2_full = nc.dram_tensor("t2_full", [NCORES * NR, T2_COLS], BF16, addr_space="Shared")
    er_loc = nc.dram_tensor("er_loc", [NR, ER_COLS], FP32)

    EXP = mybir.ActivationFunctionType.Exp
    RELU = mybir.ActivationFunctionType.Relu
    AL = mybir.AluOpType

    with tile.TileContext(nc) as tc:
        with tc.tile_pool(name="const", bufs=1) as pc, \
             tc.tile_pool(name="persist", bufs=1) as pp, \
             tc.tile_pool(name="work", bufs=4) as pw:

            iota_sb = pc.tile([P, P], BF16)
            nc.sync.dma_start(out=iota_sb[:], in_=iota_in[:])
            ident = pc.tile([P, P], FP32)
            make_identity(nc, ident[:])
            ident_bf = pc.tile([P, P], BF16)
            make_identity(nc, ident_bf[:])
            wp1_sb = pc.tile([P, 2, F1 + 8], FP32)
            nc.sync.dma_start(out=wp1_sb[:], in_=wp1.rearrange("c p f -> p c f"))
            wp2_sb = pc.tile([P, 2, F2 + 2 + F2], BF16)
            nc.gpsimd.dma_start(out=wp2_sb[:], in_=wp2.rearrange("c p f -> p c f"))
            b1_sb = pc.tile([P, F1], FP32)
            nc.sync.dma_start(out=b1_sb[:], in_=b1_in[:])
            b2_sb = pc.tile([P, F2], FP32)
            nc.sync.dma_start(out=b2_sb[:], in_=b2_in[:])
            ones_bf = pc.tile([P, 1], BF16)
            nc.vector.memset(ones_bf[:], 1.0)
            idxm_sb = pc.tile([P, NW * KT * 8], I16)
            nc.sync.dma_start(out=idxm_sb[:], in_=idxm_in[:])
            idxe_sb = pc.tile([P, NW * KT * 8], I16)
            nc.sync.dma_start(out=idxe_sb[:], in_=idxe_in[:])
            dstw_sb = pc.tile([P, NW * KT], FP32)
            nc.sync.dma_start(out=dstw_sb[:], in_=dstw_in[:])

            h_sb = pp.tile([P, NW, F1], BF16)
            resid_sb = pp.tile([P, NW, F2], FP32)

            reg8 = nc.gpsimd.to_reg(8 * P)
            regs = {}
            for k in {min(8, K_LO), K_LO % 8 or 8, min(8, KT - K_LO),
                      (KT - K_LO) % 8 or 8, min(8, KT), KT % 8 or 8}:
                if k != 8:
                    regs[k] = nc.gpsimd.to_reg(k * P)

            for _rep in range(REPS):
                # ---------------- phase A: local tables for layer 1 ----------
                with tc.tile_pool(name="pa", bufs=3) as pa, \
                     tc.tile_pool(name="px", bufs=1) as px, \
                     tc.tile_pool(name="psA", bufs=2, space="PSUM") as psA:
                    xt_sb = px.tile([P, 2, NR], FP32)
                    nc.sync.dma_start(out=xt_sb[:, 0, :], in_=xt[0])
                    nc.sync.dma_start(out=xt_sb[:, 1, :], in_=xt[1])
                    for t in range(NA):
                        ps = psA.tile([P, F1 + 8], FP32)
                        for c in range(2):
                            nc.tensor.matmul(ps[:], lhsT=xt_sb[:, c, bass.ts(t, P)],
                                             rhs=wp1_sb[:, c, :],
                                             start=(c == 0), stop=(c == 1))
                        st1 = pa.tile([P, T1_COLS], BF16)
                        nc.vector.tensor_copy(out=st1[:, 0:F1 + 4], in_=ps[:, 0:F1 + 4])
                        nc.gpsimd.memset(st1[:, F1 + 4:T1_COLS], 0.0)
                        nc.sync.dma_start(out=t1_loc[bass.ts(t, P), :], in_=st1[:])
                        ser = pa.tile([P, ER_COLS], FP32)
                        nc.vector.tensor_copy(out=ser[:, 0:4], in_=ps[:, F1 + 4:F1 + 8])
                        nc.vector.memset(ser[:, 4:ER_COLS], 0.0)
                        nc.sync.dma_start(out=er_loc[bass.ts(t, P), :], in_=ser[:])

                if PHASES >= 2:
                    if SIM:
                        nc.sync.dma_start(out=t1_full[0:NR, :], in_=t1_loc[:])
                    else:
                        nc.gpsimd.collective_compute(
                            "AllGather", AL.bypass, replica_groups=[list(range(NCORES))],
                            ins=[t1_loc[:]], outs=[t1_full[:]])

                # ---------------- phase B: layer-1 edge aggregation ----------
                with tc.tile_pool(name="pg", bufs=3) as pg, \
                     tc.tile_pool(name="_noop", bufs=1) as _noop_pool, \
                     tc.tile_pool(name="pgs", bufs=3) as pgs, \
                     tc.tile_pool(name="per", bufs=3) as per, \
                     tc.tile_pool(name="pm", bufs=8) as pm, \
                     tc.tile_pool(name="psB", bufs=3, space="PSUM") as psB:
                    for w in range(NW if PHASES >= 3 else 0):
                        col0 = w * KT * 8
                        G = pg.tile([P, KT, T1_COLS], BF16)
                        _gather_chunks(nc, G, t1_full[0:meta["HALF"], :], idxm_sb,
                                       col0, 0, K_LO, T1_COLS, reg8, regs)
                        _gather_chunks(nc, G, t1_full[meta["HALF"]:, :], idxm_sb,
                                       col0, K_LO, KT - K_LO, T1_COLS, reg8, regs)
                        ER = per.tile([P, KT, ER_COLS], FP32)
                        _gather_chunks(nc, ER, er_loc[:], idxe_sb,
                                       col0, 0, KT, ER_COLS, reg8, regs)

                        if BSUB < 2:
                            continue
                        # ex = exp(leakyrelu(el_src + er_dst)), batched per window
                        el_f = pw.tile([P, KT, HEADS1], FP32)
                        nc.vector.tensor_copy(out=el_f[:], in_=G[:, :, F1:F1 + 4])
                        e_lin = pw.tile([P, KT, HEADS1], FP32)
                        nc.vector.tensor_add(out=e_lin[:], in0=el_f[:], in1=ER[:, :, 0:4])
                        e_s = pw.tile([P, KT, HEADS1], FP32)
                        nc.vector.tensor_scalar_mul(out=e_s[:], in0=e_lin[:], scalar1=NEG_SLOPE)
                        e_lr = pw.tile([P, KT, HEADS1], FP32)
                        nc.vector.tensor_tensor(out=e_lr[:], in0=e_lin[:], in1=e_s[:], op=AL.max)
                        ex_b = pw.tile([P, KT, HEADS1], BF16)
                        nc.scalar.activation(out=ex_b[:], in_=e_lr[:], func=EXP)

                        Gs = pgs.tile([P, KT, F1], BF16)
                        nc.vector.tensor_tensor(
                            out=Gs[:].rearrange("p k (h d) -> p k h d", d=HIDDEN),
                            in0=G[:, :, 0:F1].rearrange("p k (h d) -> p k h d", d=HIDDEN),
                            in1=ex_b[:].to_broadcast([P, KT, HEADS1, HIDDEN]),
                            op=AL.mult)

                        if BSUB < 3:
                            continue
                        ps = psB.tile([P, F1], FP32)
                        ps_s = psB.tile([P, HEADS1], FP32)
                        for t in range(KT):
                            mask = pm.tile([P, P], BF16)
                            nc.vector.tensor_scalar(
                                out=mask[:], in0=iota_sb[:],
                                scalar1=dstw_sb[:, w * KT + t:w * KT + t + 1],
                                scalar2=None, op0=AL.is_equal)
                            nc.tensor.matmul(ps[:], lhsT=mask[:], rhs=Gs[:, t, :],
                                             start=(t == 0), stop=(t == KT - 1))
                            nc.tensor.matmul(ps_s[:], lhsT=mask[:], rhs=ex_b[:, t, :],
                                             start=(t == 0), stop=(t == KT - 1))

                        if BSUB < 4:
                            continue
                        s_f = pw.tile([P, HEADS1], FP32)
                        nc.vector.tensor_scalar_max(out=s_f[:], in0=ps_s[:],
                                                    scalar1=1e-30)
                        rs = pw.tile([P, HEADS1], FP32)
                        nc.vector.reciprocal(out=rs[:], in_=s_f[:])
                        hx = pw.tile([P, F1], FP32)
                        nc.vector.tensor_tensor(
                            out=hx[:].rearrange("p (h d) -> p h d", d=HIDDEN),
                            in0=ps[:].rearrange("p (h d) -> p h d", d=HIDDEN),
                            in1=rs[:].to_broadcast([P, HEADS1, HIDDEN]), op=AL.mult)
                        hb = pw.tile([P, F1], FP32)
                        nc.vector.tensor_add(out=hb[:], in0=hx[:], in1=b1_sb[:])
                        # elu(x) = relu(x) + min(exp(min(x,0)) - 1, 0)
                        xm = pw.tile([P, F1], FP32)
                        nc.vector.tensor_scalar_min(out=xm[:], in0=hb[:], scalar1=0.0)
                        xe = pw.tile([P, F1], FP32)
                        nc.scalar.activation(out=xe[:], in_=xm[:], func=EXP)
                        em = pw.tile([P, F1], FP32)
                        nc.vector.tensor_scalar(out=em[:], in0=xe[:], scalar1=-1.0,
                                                scalar2=0.0, op0=AL.add, op1=AL.min)
                        xp = pw.tile([P, F1], FP32)
                        nc.scalar.activation(out=xp[:], in_=hb[:], func=RELU)
                        nc.vector.tensor_add(out=h_sb[:, w, :], in0=em[:], in1=xp[:])

                # ---------------- phase C: local tables for layer 2 ----------
                with tc.tile_pool(name="pcw", bufs=3) as pcw, \
                     tc.tile_pool(name="psC", bufs=2, space="PSUM") as psC:
                    for w in range(NW if PHASES >= 4 else 0):
                        hT = pcw.tile([P, 2, P], BF16)
                        for c in range(2):
                            tp = psC.tile([P, P], BF16)
                            nc.tensor.transpose(out=tp[:], in_=h_sb[:, w, bass.ts(c, P)],
                                                identity=ident_bf[:])
                            nc.vector.tensor_copy(out=hT[:, c, :], in_=tp[:])
                        f2 = psC.tile([P, F2 + 2 + F2], FP32)
                        for c in range(2):
                            nc.tensor.matmul(f2[:], lhsT=hT[:, c, :], rhs=wp2_sb[:, c, :],
                                             start=(c == 0), stop=(c == 1))
                        st2 = pcw.tile([P, T2_COLS], BF16)
                        nc.vector.tensor_copy(out=st2[:, 0:F2 + 1], in_=f2[:, 0:F2 + 1])
                        nc.gpsimd.memset(st2[:, F2 + 1:T2_COLS], 0.0)
                        nc.sync.dma_start(out=t2_loc[w * 127:w * 127 + 127, :],
                                          in_=st2[0:127, :])
                        er2 = pcw.tile([P, 1], FP32)
                        nc.vector.tensor_copy(out=er2[:], in_=f2[:, F2 + 1:F2 + 2])
                        nc.sync.dma_start(out=er_loc[w * 127:w * 127 + 127, 4:5],
                                          in_=er2[0:127, :])
                        nc.vector.tensor_copy(out=resid_sb[:, w, :],
                                              in_=f2[:, F2 + 2:F2 + 2 + F2])

                if NR > NW * 127 and PHASES >= 4:
                    with tc.tile_pool(name="pz", bufs=1) as pz:
                        zt = pz.tile([P, T2_COLS], BF16)
                        nc.gpsimd.memset(zt[:], 0.0)
                        nc.sync.dma_start(out=t2_loc[NW * 127:NR, :],
                                          in_=zt[0:NR - NW * 127, :])
                if PHASES >= 4:
                    if SIM:
                        nc.sync.dma_start(out=t2_full[0:NR, :], in_=t2_loc[:])
                    else:
                        nc.gpsimd.collective_compute(
                            "AllGather", AL.bypass, replica_groups=[list(range(NCORES))],
                            ins=[t2_loc[:]], outs=[t2_full[:]])

                # ---------------- phase D: layer-2 edge aggregation ----------
                with tc.tile_pool(name="pg2", bufs=3) as pg2, \
                     tc.tile_pool(name="per2", bufs=3) as per2, \
                     tc.tile_pool(name="pm2", bufs=8) as pm2, \
                     tc.tile_pool(name="psD", bufs=3, space="PSUM") as psD:
                    for w in range(NW if PHASES >= 5 else 0):
                        col0 = w * KT * 8
                        G2 = pg2.tile([P, KT, T2_COLS], BF16)
                        _gather_chunks(nc, G2, t2_full[0:meta["HALF"], :], idxm_sb,
                                       col0, 0, K_LO, T2_COLS, reg8, regs)
                        _gather_chunks(nc, G2, t2_full[meta["HALF"]:, :], idxm_sb,
                                       col0, K_LO, KT - K_LO, T2_COLS, reg8, regs)
                        ER2 = per2.tile([P, KT, ER_COLS], FP32)
                        _gather_chunks(nc, ER2, er_loc[:], idxe_sb,
                                       col0, 0, KT, ER_COLS, reg8, regs)

                        el2 = pw.tile([P, KT, 1], FP32)
                        nc.vector.tensor_copy(out=el2[:], in_=G2[:, :, F2:F2 + 1])
                        e2 = pw.tile([P, KT, 1], FP32)
                        nc.vector.tensor_add(out=e2[:], in0=el2[:], in1=ER2[:, :, 4:5])
                        e2s = pw.tile([P, KT, 1], FP32)
                        nc.vector.tensor_scalar_mul(out=e2s[:], in0=e2[:], scalar1=NEG_SLOPE)
                        e2l = pw.tile([P, KT, 1], FP32)
                        nc.vector.tensor_tensor(out=e2l[:], in0=e2[:], in1=e2s[:], op=AL.max)
                        ex2 = pw.tile([P, KT, 1], FP32)
                        nc.scalar.activation(out=ex2[:], in_=e2l[:], func=EXP)

                        ps2 = psD.tile([P, F2], FP32)
                        ps2_s = psD.tile([P, 1], FP32)
                        for t in range(KT):
                            maskx = pm2.tile([P, P], BF16)
                            nc.vector.tensor_scalar(
                                out=maskx[:], in0=iota_sb[:],
                                scalar1=dstw_sb[:, w * KT + t:w * KT + t + 1],
                                scalar2=ex2[:, t, :], op0=AL.is_equal, op1=AL.mult)
                            nc.tensor.matmul(ps2[:], lhsT=maskx[:], rhs=G2[:, t, 0:F2],
                                             start=(t == 0), stop=(t == KT - 1))
                            nc.tensor.matmul(ps2_s[:], lhsT=maskx[:], rhs=ones_bf[:],
                                             start=(t == 0), stop=(t == KT - 1))

                        s2 = pw.tile([P, 1], FP32)
                        nc.vector.tensor_scalar_max(out=s2[:], in0=ps2_s[:],
                                                    scalar1=1e-30)
                        rs2 = pw.tile([P, 1], FP32)
                        nc.vector.reciprocal(out=rs2[:], in_=s2[:])
                        o1 = pw.tile([P, F2], FP32)
                        nc.vector.tensor_scalar_mul(out=o1[:], in0=ps2[:],
                                                    scalar1=rs2[:])
                        o2 = pw.tile([P, F2], FP32)
                        nc.vector.tensor_add(out=o2[:], in0=o1[:], in1=resid_sb[:, w, :])
                        o3 = pw.tile([P, F2], FP32)
                        nc.vector.tensor_add(out=o3[:], in0=o2[:], in1=b2_sb[:])
                        nc.sync.dma_start(out=out_loc[w * 127:w * 127 + 127, :],
                                          in_=o3[0:127, :])

    _finalize(nc)
    return nc


# --------------------------------------------------------------------------
# public entry point
# --------------------------------------------------------------------------

def prepare(x, W1, aL1, aR1, b1, W2, aL2, aR2, b2, resW2, src, dst):
    """Host prep: returns (nc, in_maps, meta). Exposed for simulator tests."""
    x = np.asarray(x, np.float32)
    n_nodes = x.shape[0]
    src = np.asarray(src, np.int64)
    dst = np.asarray(dst, np.int64)

    meta, idxm, idxe, dstw = _prep_graph(src, dst, n_nodes)
    NR, NW, KT = meta["NR"], meta["NW"], meta["KT"]
    bounds = meta["bounds"]

    # weight packing (host): WA = W1 @ blockdiag(a)
    W1 = np.asarray(W1, np.float32)
    W1r = W1.reshape(IN_DIM, HEADS1, HIDDEN)
    WA_L = np.einsum("ihd,hd->ih", W1r, np.asarray(aL1, np.float32))
    WA_R = np.einsum("ihd,hd->ih", W1r, np.asarray(aR1, np.float32))
    wp1 = np.concatenate([W1, WA_L, WA_R], axis=1).astype(np.float32)      # [256, 264]
    wp1 = wp1.reshape(2, P, F1 + 8)

    W2 = np.asarray(W2, np.float32)
    WA_L2 = (W2.reshape(F1, 1, N_CLASSES) * np.asarray(aL2, np.float32)[None]).sum(-1)
    WA_R2 = (W2.reshape(F1, 1, N_CLASSES) * np.asarray(aR2, np.float32)[None]).sum(-1)
    wp2 = np.concatenate([W2, WA_L2, WA_R2, np.asarray(resW2, np.float32)],
                         axis=1).astype(np.float32)                         # [256, 130]
    wp2 = wp2.reshape(2, P, F2 + 2 + F2)

    iota_bf = np.tile(np.arange(P, dtype=np.float32), (P, 1)).astype(ml_dtypes.bfloat16)
    b1_bc = np.tile(np.asarray(b1, np.float32)[None, :], (P, 1))
    b2_bc = np.tile(np.asarray(b2, np.float32)[None, :], (P, 1))

    in_maps = []
    for c in range(NCORES):
        n0, n1 = bounds[c], bounds[c + 1]
        xt = np.zeros((2, P, NR), np.float32)
        xloc = np.ascontiguousarray(x[n0:n1].T)          # [256, nloc]
        xt[0, :, :n1 - n0] = xloc[:P]
        xt[1, :, :n1 - n0] = xloc[P:]
        in_maps.append({
            "xt": xt, "wp1": wp1, "wp2": wp2, "iota": iota_bf,
            "b1bc": b1_bc, "b2bc": b2_bc,
            "idxm": idxm[c], "idxe": idxe[c],
            "dstw": dstw[c],
        })

    nc = _build(meta)
    return nc, in_maps, meta


def assemble(meta, per_core_out, n_nodes):
    out = np.zeros((n_nodes, F2), np.float32)
    for c in range(NCORES):
        n0, n1 = meta["bounds"][c], meta["bounds"][c + 1]
        out[n0:n1] = per_core_out[c][0:n1 - n0]
    return out


def kernel(x, W1, aL1, aR1, b1, W2, aL2, aR2, b2, resW2, src, dst,
           _trace=False):
    nc, in_maps, meta = prepare(x, W1, aL1, aR1, b1, W2, aL2, aR2, b2,
                                resW2, src, dst)
    res = run_bass_kernel_spmd(nc, in_maps, list(range(NCORES)), trace=_trace)
    out = assemble(meta, [res.results[c]["out"] for c in range(NCORES)],
                   np.asarray(x).shape[0])
    if _trace:
        return out, res
    return out



# revision 19
# speedup vs baseline: 1.4435x; 1.4435x over previous
"""Self-contained 8-core Trainium2 Bass kernel for the 2-layer GAT problem.

Strategy (v2)
-------------
* Sort edges by dst; partition edge list across 8 cores at dst-node
  boundaries, so every dst node's incoming edges live on exactly one core.
  Edge softmax and aggregation are then fully core-local.
* Phase A (per core): feat1 = x @ W1 (bf16) for the core's node block;
  el1 from the same stationary operand via host-packed WA_L columns.
  Rows [feat1|el1] go to a bf16 table, AllGathered so every core can
  gather arbitrary source rows. er1 columns go to a tiny local buffer
  (dst-side values never need gathering).
* Phase B+C (per dst-window of 127 nodes, interleaved): two merged
  dma_gather calls (lo/hi table halves, int16 row ids) pull the per-edge
  source rows. er_dst is broadcast to edge slots on the tensor engine via
  transposed one-hot masks (built from a host-replicated int8 table), so
  no per-edge er gather exists. ex = exp(prelu(el+er)) runs on the ACT
  engine and is written into the staging tile next to the scaled features,
  so one matmul per tile aggregates messages AND softmax denominators.
  Masks build on Pool (B) to balance engines. h = elu(rst/s + b1) feeds
  phase C (feat2/el2/er2/resid) immediately per window.
* Phase D: same as B for layer 2 (single head; ex folds into the mask,
  denominator via a ones-column baked into the t2 table rows).
* Output rows are written per-core and concatenated on the host.

The container's walrus rejects instructions with >1 sync-wait, and plain
Bass+Tile never inserts GPSIMD library loads - `_finalize` patches both.
"""

import math
import numpy as np
import ml_dtypes

import concourse.bass as bass
import concourse.mybir as mybir
import concourse.tile as tile
from concourse.bass_utils import run_bass_kernel_spmd
from concourse.masks import make_identity

FP32 = mybir.dt.float32
BF16 = mybir.dt.bfloat16
I16 = mybir.dt.int16
I8 = mybir.dt.int8

NCORES = 8
P = 128
NEG_SLOPE = 0.2

# problem dims (hardcoded per contract)
IN_DIM = 256
HIDDEN = 64
HEADS1 = 4
F1 = HEADS1 * HIDDEN          # 256
N_CLASSES = 64
F2 = N_CLASSES                # 64 (single head)
T1_COLS = 384                 # 768B rows: [feat1 bf16 256 | el1 bf16 4 | junk]
T1_USED = F1 + HEADS1         # 260
T2_COLS = 128                 # 256B rows: [feat2 64 | el2 | ones | junk]
T2_USED = F2 + 2              # 66


# --------------------------------------------------------------------------
# finalize passes: make Tile output compatible with this container's walrus
# --------------------------------------------------------------------------

def _insert_library_loads(nc):
    import bass_rust as _bass_rust
    from concourse.library_config import all_libraries, standard

    mask = {}
    for lib in all_libraries:
        for t in lib.instructions:
            mask[t] = mask.get(t, 0) | (1 << lib.index)
    _bass_rust.insert_library_loads(nc, mask, len(all_libraries), standard.index)


def _split_multi_waits(nc, max_waits=1):
    n = 0
    for bb in nc.m.functions[0].blocks:
        insts = bb.instructions
        if not any(i.sync_info and i.sync_info.on_wait
                   and len(i.sync_info.on_wait) > max_waits for i in insts):
            continue
        out = []
        for inst in insts:
            si = inst.sync_info
            if si and si.on_wait and len(si.on_wait) > max_waits:
                waits = list(si.on_wait)
                extra, keep = waits[:-max_waits], waits[-max_waits:]
                for j in range(0, len(extra), max_waits):
                    nop = mybir.InstNoOp(
                        name=nc.get_next_instruction_name(),
                        engine=inst.engine,
                        sync_info=mybir.SyncInfo(
                            on_wait=extra[j:j + max_waits], on_update=[]),
                        bass_nofuse=True)
                    nc.register_instruction(nop)
                    out.append(nop)
                    n += 1
                si.on_wait = keep
            out.append(inst)
        bb.instructions = out
    return n


def _finalize(nc):
    _insert_library_loads(nc)
    _split_multi_waits(nc)
    mybir.codegen_inst_isa_subclasses(nc)


# --------------------------------------------------------------------------
# host-side graph preprocessing
# --------------------------------------------------------------------------

def _wrap16(idx, ncols):
    """Wrap a 1-D index list into the [128, ncols] int16 layout dma_gather
    expects: index i at [16g + i%16, i//16] for all 8 groups g."""
    arr = np.zeros((P, ncols), np.int16)
    n = len(idx)
    cols = (n + 15) // 16
    block = np.zeros((16, ncols), np.int16)
    pad = np.zeros(cols * 16 - n, np.int16)
    w = np.concatenate([idx.astype(np.int16), pad]).reshape(cols, 16).T
    block[:, :cols] = w
    for g in range(8):
        arr[16 * g:16 * g + 16, :] = block
    return arr


def _prep_graph(src, dst, n_nodes):
    """Edge partitioning + per-core gather/mask layouts.

    The window size WS (dst nodes per window) is tuned down from 127 until
    every (core, window, table-half) fits in 8 gather tiles (1024 indices,
    the hard per-call ucode limit) so each half is ONE dma_gather call.
    """
    E = len(src)
    order = np.argsort(dst, kind="stable")
    ds = dst[order].astype(np.int64)
    ss = src[order].astype(np.int64)

    counts = np.bincount(ds, minlength=n_nodes)
    cum = np.concatenate([[0], np.cumsum(counts)])  # cum[n] = #edges dst<n

    bounds = [0]
    for i in range(1, NCORES):
        n = int(np.searchsorted(cum, round(E * i / NCORES)))
        n = min(max(n, bounds[-1] + 1), n_nodes - (NCORES - i))
        bounds.append(n)
    bounds.append(n_nodes)
    nlocs = [bounds[i + 1] - bounds[i] for i in range(NCORES)]
    NMAX = max(nlocs)

    rank_of = np.searchsorted(bounds, np.arange(n_nodes), side="right") - 1
    base_of = np.asarray(bounds[:-1])[rank_of]

    def _scan(WS, NR, HALF, kcap=None):
        table_row = rank_of * NR + (np.arange(n_nodes) - base_of)
        NW = (NMAX + WS - 1) // WS
        win_edges = []
        K_LO = K_HI = 1
        for c in range(NCORES):
            n0, n1 = bounds[c], bounds[c + 1]
            rows_c = []
            for w in range(NW):
                wn0 = n0 + WS * w
                wn1 = min(wn0 + WS, n1)
                if wn0 >= n1:
                    eds = ess = np.zeros(0, np.int64)
                else:
                    e0, e1 = cum[wn0], cum[wn1]
                    eds, ess = ds[e0:e1], ss[e0:e1]
                trow = table_row[ess] if len(ess) else np.zeros(0, np.int64)
                lo = trow < HALF
                lo_rows = trow[lo]
                hi_rows = trow[~lo] - HALF
                lo_dwin = (eds[lo] - wn0) if len(eds) else np.zeros(0, np.int64)
                hi_dwin = (eds[~lo] - wn0) if len(eds) else np.zeros(0, np.int64)
                rows_c.append((lo_rows, hi_rows, lo_dwin, hi_dwin))
                K_LO = max(K_LO, (len(lo_rows) + P - 1) // P)
                K_HI = max(K_HI, (len(hi_rows) + P - 1) // P)
                if kcap and (K_LO > kcap or K_HI > kcap):
                    return None, None, None, None
            win_edges.append(rows_c)
        return NW, win_edges, K_LO, K_HI

    WS = 128
    while True:
        WS -= 1
        NW = (NMAX + WS - 1) // WS
        NR = P * ((NW * WS + P - 1) // P)
        HALF = (NCORES // 2) * NR
        NW, win_edges, K_LO, K_HI = _scan(WS, NR, HALF, kcap=8)
        if NW is not None:
            break
        assert WS > 64, "no window size found with <=8 tiles per half"

    KT = K_LO + K_HI
    idxm = np.zeros((NCORES, P, NW * KT * 8), np.int16)
    dstw = np.full((NCORES, P, NW * KT), 127.0, np.float32)

    for c in range(NCORES):
        for w in range(NW):
            lo_rows, hi_rows, lo_dwin, hi_dwin = win_edges[c][w]
            col0 = w * KT * 8
            idxm[c][:, col0:col0 + K_LO * 8] = _wrap16(
                np.pad(lo_rows, (0, K_LO * P - len(lo_rows))), K_LO * 8)
            idxm[c][:, col0 + K_LO * 8:col0 + KT * 8] = _wrap16(
                np.pad(hi_rows, (0, K_HI * P - len(hi_rows))), K_HI * 8)
            # per-slot in-window dst (127 = trash for pads)
            dw = np.full(KT * P, 127, np.int64)
            dw[:len(lo_dwin)] = lo_dwin
            dw[K_LO * P:K_LO * P + len(hi_dwin)] = hi_dwin
            s = np.arange(KT * P)
            dstw[c][s % P, w * KT + s // P] = dw

    # transposed-broadcast dst table (int8, row-replicated) for maskT builds
    dstwT8 = np.empty((NCORES, P, NW * KT * P), np.int8)
    for c in range(NCORES):
        flat = dstw[c].T.reshape(1, NW * KT * P).astype(np.int8)
        dstwT8[c][:] = flat  # broadcast to all 128 partitions

    meta = dict(NW=NW, NR=NR, KT=KT, K_LO=K_LO, K_HI=K_HI, HALF=HALF, WS=WS,
                bounds=bounds, nlocs=nlocs)
    return meta, idxm, dstw, dstwT8


# --------------------------------------------------------------------------
# device program
# --------------------------------------------------------------------------

def _gather_chunks(nc, out_tile, in_ap, idx_sb, col8, t0, ntiles, elem, regs):
    """Issue dma_gather calls of at most GCAP tiles each."""
    import os
    gcap = int(os.environ.get("GAT_GCAP", "8"))
    done = 0
    while done < ntiles:
        k = min(gcap, ntiles - done)
        nc.gpsimd.dma_gather(
            out_ap=out_tile[:, t0 + done:t0 + done + k, :], in_ap=in_ap,
            idxs_ap=idx_sb[:, col8 + (t0 + done) * 8:col8 + (t0 + done + k) * 8],
            num_idxs=k * P, num_idxs_reg=regs[k], elem_size=elem)
        done += k


def _build(meta):
    import os
    PHASES = int(os.environ.get("GAT_PHASES", "5"))
    REPS = int(os.environ.get("GAT_REPS", "1"))
    SIM = int(os.environ.get("GAT_SIM", "0"))
    DBG = int(os.environ.get("GAT_DBG", "0"))
    NW, NR, KT, K_LO = meta["NW"], meta["NR"], meta["KT"], meta["K_LO"]
    K_HI = meta["K_HI"]
    WS = meta["WS"]
    NA = NR // P

    nc = bass.Bass(num_devices=NCORES)
    xt = nc.declare_dram_parameter("xt", [2, P, NR], BF16, isOutput=False)
    wp1 = nc.declare_dram_parameter("wp1", [2, P, F1 + 8], BF16, isOutput=False)
    wp2 = nc.declare_dram_parameter("wp2", [2, P, F2 + 2 + F2], BF16, isOutput=False)
    iota_in = nc.declare_dram_parameter("iota", [P, P], BF16, isOutput=False)
    iota8_in = nc.declare_dram_parameter("iota8", [P, 1], FP32, isOutput=False)
    iotaw_in = nc.declare_dram_parameter("iotaw", [P, KT * P], BF16, isOutput=False)
    b1_in = nc.declare_dram_parameter("b1bc", [P, F1], FP32, isOutput=False)
    b2_in = nc.declare_dram_parameter("b2bc", [P, F2], FP32, isOutput=False)
    idxm_in = nc.declare_dram_parameter("idxm", [P, NW * KT * 8], I16, isOutput=False)
    dstw_in = nc.declare_dram_parameter("dstw", [P, NW * KT], FP32, isOutput=False)
    dstwT8_in = nc.declare_dram_parameter("dstwT8", [P, NW * KT * P], I8,
                                          isOutput=False)
    out_loc = nc.declare_dram_parameter("out", [NR, F2], FP32, isOutput=True)
    dbg_h = (nc.declare_dram_parameter("dbg_h", [NR, F1], BF16, isOutput=True)
             if DBG else None)

    t1_loc = nc.dram_tensor("t1_loc", [NR, T1_COLS], BF16)
    t1_full = nc.dram_tensor("t1_full", [NCORES * NR, T1_COLS], BF16,
                             addr_space="Shared")
    t2_loc = nc.dram_tensor("t2_loc", [NR, T2_COLS], BF16)
    t2_full = nc.dram_tensor("t2_full", [NCORES * NR, T2_COLS], BF16,
                             addr_space="Shared")
    er1_dram = nc.dram_tensor("er1_dram", [NR, HEADS1], BF16)
    er2_dram = nc.dram_tensor("er2_dram", [NR, 1], BF16)

    EXP = mybir.ActivationFunctionType.Exp
    RELU = mybir.ActivationFunctionType.Relu
    PRELU = mybir.ActivationFunctionType.Prelu
    COPY = mybir.ActivationFunctionType.Copy
    AL = mybir.AluOpType

    with tile.TileContext(nc) as tc:
        with tc.tile_pool(name="const", bufs=1) as pc, \
             tc.tile_pool(name="persist", bufs=1) as pp, \
             tc.tile_pool(name="work", bufs=4) as pw:

            iota_sb = pc.tile([P, P], BF16)
            nc.sync.dma_start(out=iota_sb[:], in_=iota_in[:])
            iota8_sb = pc.tile([P, 1], FP32)
            nc.sync.dma_start(out=iota8_sb[:], in_=iota8_in[:])
            iotaw_sb = pc.tile([P, KT, P], BF16)
            nc.sync.dma_start(out=iotaw_sb[:], in_=iotaw_in[:])
            ident_bf = pc.tile([P, P], BF16)
            make_identity(nc, ident_bf[:])
            wp1_sb = pc.tile([P, 2, F1 + 8], BF16)
            nc.sync.dma_start(out=wp1_sb[:], in_=wp1.rearrange("c p f -> p c f"))
            wp2_sb = pc.tile([P, 2, F2 + 2 + F2], BF16)
            nc.gpsimd.dma_start(out=wp2_sb[:], in_=wp2.rearrange("c p f -> p c f"))
            b1_sb = pc.tile([P, F1], FP32)
            nc.sync.dma_start(out=b1_sb[:], in_=b1_in[:])
            b2_sb = pc.tile([P, F2], FP32)
            nc.sync.dma_start(out=b2_sb[:], in_=b2_in[:])
            idxm_sb = pc.tile([P, NW * KT * 8], I16)
            nc.sync.dma_start(out=idxm_sb[:], in_=idxm_in[:])
            dstw_sb = pc.tile([P, NW * KT], FP32)
            nc.sync.dma_start(out=dstw_sb[:], in_=dstw_in[:])

            er1_sb = pp.tile([P, NW, HEADS1], BF16)
            er2_sb = pp.tile([P, NW], BF16)
            resid_sb = pp.tile([P, NW, F2], FP32)

            gcap = int(os.environ.get("GAT_GCAP", "8"))
            needed = set()
            for n in (K_LO, K_HI):
                left = n
                while left > 0:
                    k = min(gcap, left)
                    needed.add(k)
                    left -= k
            regs = {k: nc.gpsimd.to_reg(k * P) for k in needed}

            for _rep in range(REPS):
                # ---------------- phase A: layer-1 node table -----------------
                with tc.tile_pool(name="pa", bufs=3) as pa, \
                     tc.tile_pool(name="px", bufs=1) as px, \
                     tc.tile_pool(name="psA", bufs=2, space="PSUM") as psA:
                    xt_sb = px.tile([P, 2, NR], BF16)
                    nc.sync.dma_start(out=xt_sb[:, 0, :], in_=xt[0])
                    nc.sync.dma_start(out=xt_sb[:, 1, :], in_=xt[1])
                    for t in range(NA):
                        ps = psA.tile([P, F1 + 8], FP32)
                        for c in range(2):
                            nc.tensor.matmul(ps[:], lhsT=xt_sb[:, c, bass.ts(t, P)],
                                             rhs=wp1_sb[:, c, :],
                                             start=(c == 0), stop=(c == 1))
                        st1 = pa.tile([P, T1_USED], BF16)
                        nc.scalar.activation(out=st1[:], in_=ps[:, 0:T1_USED],
                                             func=COPY)
                        ster = pa.tile([P, HEADS1], BF16)
                        nc.vector.tensor_copy(out=ster[:], in_=ps[:, F1 + 4:F1 + 8])
                        nc.sync.dma_start(out=t1_loc[bass.ts(t, P), 0:T1_USED],
                                          in_=st1[:])
                        nc.sync.dma_start(out=er1_dram[bass.ts(t, P), :], in_=ster[:])

                # er1 window-layout load (partition 127 zeroed: trash-slot er)
                nc.vector.memset(er1_sb[:], 0.0)
                nc.sync.dma_start(
                    out=er1_sb[0:WS, :, :],
                    in_=er1_dram[0:NW * WS, :].rearrange("(w d) h -> d w h", d=WS))

                if PHASES >= 2:
                    if SIM:
                        nc.sync.dma_start(out=t1_full[0:NR, :], in_=t1_loc[:])
                    else:
                        nc.gpsimd.collective_compute(
                            "AllGather", AL.bypass,
                            replica_groups=[list(range(NCORES))],
                            ins=[t1_loc[:]], outs=[t1_full[:]])

                # ---------- phase B+C: layer-1 aggregation + layer-2 table ----
                with tc.tile_pool(name="pg", bufs=3) as pg, \
                     tc.tile_pool(name="pgs", bufs=3) as pgs, \
                     tc.tile_pool(name="pdt", bufs=3) as pdt, \
                     tc.tile_pool(name="pm", bufs=2) as pm, \
                     tc.tile_pool(name="pmT", bufs=2) as pmT, \
                     tc.tile_pool(name="pcw", bufs=3) as pcw, \
                     tc.tile_pool(name="psE", bufs=2, space="PSUM") as psE, \
                     tc.tile_pool(name="psB", bufs=2, space="PSUM") as psB, \
                     tc.tile_pool(name="psT", bufs=2, space="PSUM") as psT, \
                     tc.tile_pool(name="psC", bufs=2, space="PSUM") as psC:
                    for w in range(NW if PHASES >= 3 else 0):
                        col8 = w * KT * 8
                        G = pg.tile([P, KT, T1_COLS], BF16)
                        _gather_chunks(nc, G, t1_full[0:meta["HALF"], :], idxm_sb,
                                       col8, 0, K_LO, T1_COLS, regs)
                        _gather_chunks(nc, G, t1_full[meta["HALF"]:, :], idxm_sb,
                                       col8, K_LO, K_HI, T1_COLS, regs)
                        dT = pdt.tile([P, KT, P], I8)
                        nc.sync.dma_start(
                            out=dT[:], in_=dstwT8_in[:, w * KT * P:(w + 1) * KT * P])

                        # er broadcast to slots via transposed masks (PE)
                        mT_all = pmT.tile([P, KT, P], BF16)
                        nc.vector.tensor_scalar(
                            out=mT_all[:], in0=dT[:], scalar1=iota8_sb[:],
                            scalar2=None, op0=AL.is_equal)
                        ps_er = psE.tile([P, KT, HEADS1], FP32)
                        for t in range(KT):
                            nc.tensor.matmul(ps_er[:, t, :], lhsT=mT_all[:, t, :],
                                             rhs=er1_sb[:, w, :],
                                             start=True, stop=True)

                        # ex = exp(prelu(el+er)) -> staging cols next to Gs
                        Gse = pgs.tile([P, KT, T1_USED], BF16)
                        e_lin = pw.tile([P, KT, HEADS1], FP32)
                        nc.vector.tensor_add(out=e_lin[:], in0=G[:, :, F1:F1 + 4],
                                             in1=ps_er[:])
                        e_lr = pw.tile([P, KT, HEADS1], FP32)
                        nc.scalar.activation(out=e_lr[:], in_=e_lin[:], func=PRELU,
                                             alpha=NEG_SLOPE)
                        nc.scalar.activation(out=Gse[:, :, F1:F1 + 4], in_=e_lr[:],
                                             func=EXP)
                        nc.vector.tensor_tensor(
                            out=Gse[:, :, 0:F1].rearrange(
                                "p k (h d) -> p k h d", d=HIDDEN),
                            in0=G[:, :, 0:F1].rearrange(
                                "p k (h d) -> p k h d", d=HIDDEN),
                            in1=Gse[:, :, F1:F1 + 4].to_broadcast(
                                [P, KT, HEADS1, HIDDEN]),
                            op=AL.mult)

                        # aggregation: one matmul per tile covers msgs + denom
                        mask_all = pm.tile([P, KT, P], BF16)
                        nc.vector.tensor_tensor(
                            out=mask_all[:], in0=iotaw_sb[:],
                            in1=dstw_sb[:, w * KT:(w + 1) * KT].to_broadcast(
                                [P, KT, P]),
                            op=AL.is_equal)
                        ps = psB.tile([P, T1_USED], FP32)
                        for t in range(KT):
                            nc.tensor.matmul(ps[:], lhsT=mask_all[:, t, :],
                                             rhs=Gse[:, t, :],
                                             start=(t == 0), stop=(t == KT - 1))

                        # h = elu(rst/s + b1)
                        s_f = pw.tile([P, HEADS1], FP32)
                        nc.vector.tensor_scalar_max(out=s_f[:], in0=ps[:, F1:F1 + 4],
                                                    scalar1=1e-30)
                        rs = pw.tile([P, HEADS1], FP32)
                        nc.vector.reciprocal(out=rs[:], in_=s_f[:])
                        hx = pw.tile([P, F1], FP32)
                        nc.vector.tensor_tensor(
                            out=hx[:].rearrange("p (h d) -> p h d", d=HIDDEN),
                            in0=ps[:, 0:F1].rearrange("p (h d) -> p h d", d=HIDDEN),
                            in1=rs[:].to_broadcast([P, HEADS1, HIDDEN]), op=AL.mult)
                        hb = pw.tile([P, F1], FP32)
                        nc.vector.tensor_add(out=hb[:], in0=hx[:], in1=b1_sb[:])
                        xm = pw.tile([P, F1], FP32)
                        nc.vector.tensor_scalar_min(out=xm[:], in0=hb[:], scalar1=0.0)
                        xe = pw.tile([P, F1], FP32)
                        nc.scalar.activation(out=xe[:], in_=xm[:], func=EXP)
                        em = pw.tile([P, F1], FP32)
                        nc.vector.tensor_scalar(out=em[:], in0=xe[:], scalar1=-1.0,
                                                scalar2=0.0, op0=AL.add, op1=AL.min)
                        xp = pw.tile([P, F1], FP32)
                        nc.scalar.activation(out=xp[:], in_=hb[:], func=RELU)
                        h_win = pw.tile([P, F1], BF16)
                        nc.vector.tensor_add(out=h_win[:], in0=em[:], in1=xp[:])
                        if DBG:
                            nc.sync.dma_start(out=dbg_h[w * WS:w * WS + WS, :],
                                              in_=h_win[0:WS, :])

                        # phase C for this window
                        if PHASES >= 4:
                            hT = pcw.tile([P, 2, P], BF16)
                            for c in range(2):
                                tp = psT.tile([P, P], BF16)
                                nc.tensor.transpose(out=tp[:],
                                                    in_=h_win[:, bass.ts(c, P)],
                                                    identity=ident_bf[:])
                                nc.scalar.activation(out=hT[:, c, :], in_=tp[:],
                                                     func=COPY)
                            f2 = psC.tile([P, F2 + 2 + F2], FP32)
                            for c in range(2):
                                nc.tensor.matmul(f2[:], lhsT=hT[:, c, :],
                                                 rhs=wp2_sb[:, c, :],
                                                 start=(c == 0), stop=(c == 1))
                            st2 = pcw.tile([P, T2_USED], BF16)
                            nc.scalar.activation(out=st2[:, 0:F2 + 1],
                                                 in_=f2[:, 0:F2 + 1], func=COPY)
                            nc.vector.memset(st2[:, F2 + 1:F2 + 2], 1.0)
                            nc.sync.dma_start(
                                out=t2_loc[w * WS:w * WS + WS, 0:T2_USED],
                                in_=st2[0:WS, :])
                            ster2 = pcw.tile([P, 1], BF16)
                            nc.vector.tensor_copy(out=ster2[:],
                                                  in_=f2[:, F2 + 1:F2 + 2])
                            nc.sync.dma_start(out=er2_dram[w * WS:w * WS + WS, :],
                                              in_=ster2[0:WS, :])
                            nc.vector.tensor_copy(out=resid_sb[:, w, :],
                                                  in_=f2[:, F2 + 2:F2 + 2 + F2])

                if PHASES >= 4:
                    nc.vector.memset(er2_sb[:], 0.0)
                    nc.sync.dma_start(
                        out=er2_sb[0:WS, :],
                        in_=er2_dram[0:NW * WS, :].rearrange("(w d) h -> d (w h)",
                                                             d=WS))
                    if SIM:
                        nc.sync.dma_start(out=t2_full[0:NR, :], in_=t2_loc[:])
                    else:
                        nc.gpsimd.collective_compute(
                            "AllGather", AL.bypass,
                            replica_groups=[list(range(NCORES))],
                            ins=[t2_loc[:]], outs=[t2_full[:]])

                # ---------------- phase D: layer-2 edge aggregation ----------
                with tc.tile_pool(name="pg2", bufs=3) as pg2, \
                     tc.tile_pool(name="pdt2", bufs=3) as pdt2, \
                     tc.tile_pool(name="pm2", bufs=8) as pm2, \
                     tc.tile_pool(name="pmT2", bufs=2) as pmT2, \
                     tc.tile_pool(name="psE2", bufs=3, space="PSUM") as psE2, \
                     tc.tile_pool(name="psD", bufs=2, space="PSUM") as psD:
                    for w in range(NW if PHASES >= 5 else 0):
                        col8 = w * KT * 8
                        G2 = pg2.tile([P, KT, T2_COLS], BF16)
                        _gather_chunks(nc, G2, t2_full[0:meta["HALF"], :], idxm_sb,
                                       col8, 0, K_LO, T2_COLS, regs)
                        _gather_chunks(nc, G2, t2_full[meta["HALF"]:, :], idxm_sb,
                                       col8, K_LO, K_HI, T2_COLS, regs)
                        dT2 = pdt2.tile([P, KT, P], I8)
                        nc.sync.dma_start(
                            out=dT2[:],
                            in_=dstwT8_in[:, w * KT * P:(w + 1) * KT * P])

                        mT2_all = pmT2.tile([P, KT, P], BF16)
                        nc.vector.tensor_scalar(
                            out=mT2_all[:], in0=dT2[:], scalar1=iota8_sb[:],
                            scalar2=None, op0=AL.is_equal)
                        ps_er2 = psE2.tile([P, KT, 1], FP32)
                        for t in range(KT):
                            nc.tensor.matmul(ps_er2[:, t, :], lhsT=mT2_all[:, t, :],
                                             rhs=er2_sb[:, w:w + 1],
                                             start=True, stop=True)

                        e2 = pw.tile([P, KT, 1], FP32)
                        nc.vector.tensor_add(
                            out=e2[:], in0=G2[:, :, F2:F2 + 1], in1=ps_er2[:])
                        e2l = pw.tile([P, KT, 1], FP32)
                        nc.scalar.activation(out=e2l[:], in_=e2[:], func=PRELU,
                                             alpha=NEG_SLOPE)
                        ex2 = pw.tile([P, KT, 1], FP32)
                        nc.scalar.activation(out=ex2[:], in_=e2l[:], func=EXP)

                        ps2 = psD.tile([P, T2_USED], FP32)
                        for t in range(KT):
                            maskx = pm2.tile([P, P], BF16)
                            nc.vector.tensor_scalar(
                                out=maskx[:], in0=iota_sb[:],
                                scalar1=dstw_sb[:, w * KT + t:w * KT + t + 1],
                                scalar2=ex2[:, t, :], op0=AL.is_equal,
                                op1=AL.mult)
                            nc.tensor.matmul(ps2[:], lhsT=maskx[:],
                                             rhs=G2[:, t, 0:T2_USED],
                                             start=(t == 0), stop=(t == KT - 1))

                        s2 = pw.tile([P, 1], FP32)
                        nc.vector.tensor_scalar_max(out=s2[:],
                                                    in0=ps2[:, F2 + 1:F2 + 2],
                                                    scalar1=1e-30)
                        rs2 = pw.tile([P, 1], FP32)
                        nc.vector.reciprocal(out=rs2[:], in_=s2[:])
                        o1 = pw.tile([P, F2], FP32)
                        nc.vector.scalar_tensor_tensor(
                            out=o1[:], in0=ps2[:, 0:F2], scalar=rs2[:],
                            in1=resid_sb[:, w, :], op0=AL.mult, op1=AL.add)
                        o3 = pw.tile([P, F2], FP32)
                        nc.vector.tensor_add(out=o3[:], in0=o1[:], in1=b2_sb[:])
                        nc.sync.dma_start(out=out_loc[w * WS:w * WS + WS, :],
                                          in_=o3[0:WS, :])

    _finalize(nc)
    return nc


# --------------------------------------------------------------------------
# public entry point
# --------------------------------------------------------------------------

def prepare(x, W1, aL1, aR1, b1, W2, aL2, aR2, b2, resW2, src, dst):
    """Host prep: returns (nc, in_maps, meta). Exposed for simulator tests."""
    x = np.asarray(x, np.float32)
    n_nodes = x.shape[0]
    src = np.asarray(src, np.int64)
    dst = np.asarray(dst, np.int64)

    meta, idxm, dstw, dstwT8 = _prep_graph(src, dst, n_nodes)
    NR, NW, KT = meta["NR"], meta["NW"], meta["KT"]
    bounds = meta["bounds"]

    # weight packing (host): WA = W1 @ blockdiag(a)
    W1 = np.asarray(W1, np.float32)
    W1r = W1.reshape(IN_DIM, HEADS1, HIDDEN)
    WA_L = np.einsum("ihd,hd->ih", W1r, np.asarray(aL1, np.float32))
    WA_R = np.einsum("ihd,hd->ih", W1r, np.asarray(aR1, np.float32))
    wp1 = np.concatenate([W1, WA_L, WA_R], axis=1)                 # [256, 264]
    wp1 = wp1.reshape(2, P, F1 + 8).astype(ml_dtypes.bfloat16)

    W2 = np.asarray(W2, np.float32)
    WA_L2 = (W2.reshape(F1, 1, N_CLASSES) * np.asarray(aL2, np.float32)[None]).sum(-1)
    WA_R2 = (W2.reshape(F1, 1, N_CLASSES) * np.asarray(aR2, np.float32)[None]).sum(-1)
    wp2 = np.concatenate([W2, WA_L2, WA_R2, np.asarray(resW2, np.float32)],
                         axis=1)                                   # [256, 130]
    wp2 = wp2.reshape(2, P, F2 + 2 + F2).astype(ml_dtypes.bfloat16)

    iota_bf = np.tile(np.arange(P, dtype=np.float32), (P, 1)).astype(
        ml_dtypes.bfloat16)
    iota8 = np.arange(P, dtype=np.float32).reshape(P, 1)
    iotaw = np.tile(np.tile(np.arange(P, dtype=np.float32), KT)[None, :],
                    (P, 1)).astype(ml_dtypes.bfloat16)
    b1_bc = np.tile(np.asarray(b1, np.float32)[None, :], (P, 1))
    b2_bc = np.tile(np.asarray(b2, np.float32)[None, :], (P, 1))

    in_maps = []
    for c in range(NCORES):
        n0, n1 = bounds[c], bounds[c + 1]
        xt = np.zeros((2, P, NR), ml_dtypes.bfloat16)
        xloc = np.ascontiguousarray(x[n0:n1].T).astype(ml_dtypes.bfloat16)
        xt[0, :, :n1 - n0] = xloc[:P]
        xt[1, :, :n1 - n0] = xloc[P:]
        in_maps.append({
            "xt": xt, "wp1": wp1, "wp2": wp2, "iota": iota_bf, "iota8": iota8,
            "b1bc": b1_bc, "b2bc": b2_bc,
            "idxm": idxm[c], "dstw": dstw[c], "dstwT8": dstwT8[c],
            "iotaw": iotaw,
        })

    nc = _build(meta)
    return nc, in_maps, meta


def assemble(meta, per_core_out, n_nodes):
    out = np.zeros((n_nodes, F2), np.float32)
    for c in range(NCORES):
        n0, n1 = meta["bounds"][c], meta["bounds"][c + 1]
        out[n0:n1] = per_core_out[c][0:n1 - n0]
    return out


def kernel(x, W1, aL1, aR1, b1, W2, aL2, aR2, b2, resW2, src, dst,
           _trace=False):
    nc, in_maps, meta = prepare(x, W1, aL1, aR1, b1, W2, aL2, aR2, b2,
                                resW2, src, dst)
    res = run_bass_kernel_spmd(nc, in_maps, list(range(NCORES)), trace=_trace)
    out = assemble(meta, [res.results[c]["out"] for c in range(NCORES)],
                   np.asarray(x).shape[0])
    if _trace:
        return out, res
    return out
